# revision 26
# baseline (speedup 1.0000x reference)
"""Trainium2 Bass kernel for LowRankTriLinearFusionAttn.

Math (per sample b):
  g  = relu(LN(h_g  @ Wg.T + bg))          (256)
  d2 = relu(LN(h_2d @ W2.T + b2))          (256)
  d3 = relu(LN(h_3d @ W3.T + b3))          (256)
  z_r[b,r,:] = (g U_r^T) * (d2 V_r^T) * (d3 S_r^T)     r in 0..15
  beta = softmax(relu([h_g|h_2d|h_3d] @ Wa1.T + ba1) @ Wa2.T + ba2)
  z[b,:] = sum_r beta[b,r] * z_r[b,r,:]

Sharding: pure data parallel over 8 NeuronCores (batch 8192 -> 1024/core).
Weights are replicated; the host pre-packs them transposed (contraction dim
on partitions) and cast to bf16 — standard offline model packing.
Activations are cast to bf16 by SWDGE DMA and transposed on the PE.

v5 vs the earlier baseline:
- PSUM evicts of the x^T transposes are merged per (tile, modality) into one
  full-width op ([128, KD*128], up to a whole 2KB bank) and round-robined
  across DVE/ACT/GPSIMD instead of 72 narrow ACT copies — the ACT engine was
  the hidden co-bottleneck (69% busy) and its backlog stalled LDWEIGHTS.
- All g^T transposes happen in the projection phase (6 transposes -> one
  PSUM bank -> a single merged evict), so the rank phase never waits on
  weight production.
- With identity LN affine (host-checked), relu folds into the LN-normalize
  eviction on ACT; with zero biases the ones-row bias matmuls are skipped.
- uvs is loaded in 8 rank-chunk slices and wave 0 walks chunks c-major
  (across tiles) so rank work starts while uvs is still streaming.
"""

import sys
import types

import numpy as np
import ml_dtypes

import concourse.bass as bass
import concourse.tile as tile
from concourse import bacc
from concourse import mybir
from concourse.bass import ts
from concourse.bass_utils import run_bass_kernel_spmd
import bass_rust


def _ensure_ntff_hook():
    """Provide antenv.axon_hooks if the image's antenv stub lacks it, so
    run_bass_kernel_spmd(trace=True) can capture NTFF profiles under axon."""
    try:
        import antenv.axon_hooks  # noqa: F401
        return
    except ImportError:
        pass
    try:
        from trn_agent_boot.trn_boot import _ntff_profile_via_ctypes

        hook = _ntff_profile_via_ctypes("/opt/axon/libaxon_pjrt.so")
    except Exception:
        hook = None
    mod = types.ModuleType("antenv.axon_hooks")
    _state = {"hook": hook}
    mod.get_axon_ntff_profile_hook = lambda: _state["hook"]
    mod.set_axon_ntff_profile_hook = lambda h: _state.update(hook=h)
    sys.modules["antenv.axon_hooks"] = mod


_ensure_ntff_hook()

BF16 = mybir.dt.bfloat16
F32 = mybir.dt.float32
AF = mybir.ActivationFunctionType
OP = mybir.AluOpType

N_CORES = 8
B = 8192
D_G, D_2D, D_3D = 512, 768, 1024
D_CAT = D_G + D_2D + D_3D  # 2304
D_F, RANK, ATTN_H = 256, 16, 512
RD = RANK * D_F  # 4096
P = 128

BC = B // N_CORES           # 1024 samples per core
NBT = BC // P               # 8 batch tiles per core
KD = [D_G // P, D_2D // P, D_3D // P]   # k-tiles per modality: 4, 6, 8
KOFF = [0, KD[0], KD[0] + KD[1]]        # xt chunk offsets: 0, 4, 10
NK = D_CAT // P             # 18
NH = ATTN_H // P            # 4
NCH = RD // 512             # 8 chunks of 512 in the rank-expanded dim
KF = D_F // P               # 2 k-tiles for the 256-dim contraction
EPS = 1e-5


def build_kernel(bc=BC, fast_affine=True, zero_bias=True):
    assert bc % 512 == 0
    nbt = bc // P
    nc = bacc.Bacc("TRN2", debug=False)

    # ---- external I/O (per-core shapes) ----
    h_g = nc.dram_tensor("h_g", [bc, D_G], F32, kind="ExternalInput").ap()
    h_2d = nc.dram_tensor("h_2d", [bc, D_2D], F32, kind="ExternalInput").ap()
    h_3d = nc.dram_tensor("h_3d", [bc, D_3D], F32, kind="ExternalInput").ap()
    # weights arrive pre-packed partition-major ([128, ...] contiguous per
    # partition row) so each load is 128 fat contiguous DMA descriptors
    wc_t = nc.dram_tensor("wc_t", [P, NK * D_F], BF16, kind="ExternalInput").ap()
    uvs_t = nc.dram_tensor("uvs_t", [P, 6 * RD], BF16, kind="ExternalInput").ap()
    wa1_t = nc.dram_tensor(
        "wa1_t", [P, NK * ATTN_H], BF16, kind="ExternalInput"
    ).ap()
    wa2_t = nc.dram_tensor("wa2_t", [P, NH * RANK], BF16, kind="ExternalInput").ap()
    consts_f = nc.dram_tensor("consts_f", [P, 16], F32, kind="ExternalInput").ap()
    consts_b = nc.dram_tensor("consts_b", [4, D_F], BF16, kind="ExternalInput").ap()
    ident_in = nc.dram_tensor("ident", [P, P], BF16, kind="ExternalInput").ap()
    z_out = nc.dram_tensor("z", [bc, D_F], F32, kind="ExternalOutput").ap()

    from contextlib import ExitStack

    with tile.TileContext(nc) as tc, ExitStack() as ctx:
        consts = ctx.enter_context(tc.tile_pool(name="consts", bufs=1))
        wpool = ctx.enter_context(tc.tile_pool(name="w", bufs=1))
        xtp = ctx.enter_context(tc.tile_pool(name="xt", bufs=1))
        gtp = ctx.enter_context(tc.tile_pool(name="gt", bufs=1))
        sp = ctx.enter_context(tc.tile_pool(name="sp", bufs=16))
        upool = ctx.enter_context(tc.tile_pool(name="up", bufs=2))
        zp = ctx.enter_context(tc.tile_pool(name="zacc", bufs=2))
        # phase-A PSUM pool: closed before the rank waves, whose pool (pp2,
        # opened later) uses 4KB two-bank slots for tile-paired chunks
        ppcm = tc.tile_pool(name="ps", bufs=6, space="PSUM")
        pp = ppcm.__enter__()
        xnp = tc.tile_pool(name="xn", bufs=4)     # scoped: closed after stage 1
        xnpool = xnp.__enter__()

        # ---------- constants ----------
        identity = consts.tile([P, P], BF16, tag="ident")
        nc.sync.dma_start(out=identity, in_=ident_in)
        ones_row = consts.tile([1, P], BF16, tag="ones")
        nc.vector.memset(ones_row, 1.0)
        eps_t = consts.tile([P, 1], F32, tag="eps")
        nc.vector.memset(eps_t, EPS)
        cf_sb = consts.tile([P, 16], F32, tag="cf")  # ba1(4) lnw(6) lnb(6)
        nc.sync.dma_start(out=cf_sb, in_=consts_f)
        cb_sb = consts.tile([1, 4, D_F], BF16, tag="cb")  # bg b2 b3 ba2pad
        nc.sync.dma_start(out=cb_sb, in_=consts_b.rearrange("(o m) n -> o m n", o=1))
        ba1_sb = cf_sb[:, 0:NH]
        lnw_sb = cf_sb[:, 4:10]
        lnb_sb = cf_sb[:, 10:16]
        bias_sb = cb_sb[:, 0:3, :]
        ba2_sb = cb_sb[:, 3, 0:RANK]

        # ---------- weights (sync queue; wc free, the rest dep-ordered
        # after the input cast stream so inputs win the HBM race) ----------
        wc_sb = wpool.tile([P, NK, D_F], BF16, tag="wc")
        wc_in = wc_t.rearrange("p (t n) -> p t n", t=NK)
        for m in range(3):
            sl = slice(KOFF[m], KOFF[m] + KD[m])
            nc.sync.dma_start(out=wc_sb[:, sl, :], in_=wc_in[:, sl, :])
        wa1_sb = wpool.tile([P, NK, ATTN_H], BF16, tag="wa1")
        d_wa1 = nc.sync.dma_start(
            out=wa1_sb, in_=wa1_t.rearrange("p (t n) -> p t n", t=NK)
        )
        wa2_sb = wpool.tile([P, NH, RANK], BF16, tag="wa2")
        d_wa2 = nc.sync.dma_start(
            out=wa2_sb, in_=wa2_t.rearrange("p (t n) -> p t n", t=NH)
        )
        uvs_sb = wpool.tile([P, 6, RD], BF16, tag="uvs")
        uvs_in = uvs_t.rearrange("p (t n) -> p t n", t=6)
        d_uvs = []
        for c in range(NCH):
            d_uvs.append(
                nc.sync.dma_start(
                    out=uvs_sb[:, :, ts(c, 512)], in_=uvs_in[:, :, ts(c, 512)]
                )
            )

        # ---------- input cast (per (tile, modality) SWDGE cast-DMAs,
        # tile-major so each batch tile is ready after 3 DMAs) ----------
        xin = [h_g, h_2d, h_3d]
        xn = [[None] * 3 for _ in range(nbt)]
        xn_dmas = []
        for t in range(nbt):
            for m in range(3):
                t_ = xnpool.tile(
                    [P, KD[m] * P], BF16, tag=f"xn{m}", name=f"xn{t}_{m}"
                )
                d = nc.gpsimd.dma_start(out=t_, in_=xin[m][ts(t, P), :])
                xn_dmas.append(d)
                xn[t][m] = t_
        # big weights release once tiles 0-3 are through; uvs slices follow
        bass_rust.add_dep_helper(
            d_wa1.ins, xn_dmas[3 * 4 - 1].ins, reason="wa1 after xn tile 3"
        )
        bass_rust.add_dep_helper(d_uvs[0].ins, d_wa1.ins, reason="uvs0 after wa1")
        bass_rust.add_dep_helper(d_wa2.ins, d_uvs[1].ins, reason="wa2 after uvs1")
        bass_rust.add_dep_helper(d_uvs[4].ins, d_wa2.ins, reason="uvs4 after wa2")

        # ---------- HAM warm-up: the PE clock gate opens after ~3.4us of
        # sustained activity; the first real matmuls land ~11us in (DMA
        # preamble + first casts), so burn the idle start on dummy identity
        # matmuls to enter phase A at 2.4GHz instead of 1.2 ----------
        warm = pp.tile([P, P], F32, tag="ps", name="warm")
        for _ in range(56):
            nc.tensor.matmul(warm, lhsT=identity, rhs=identity,
                             start=True, stop=True)

        # ---------- x transposes + projections + LN + gT, per tile -------
        # KD PE transposes fill one [128, KD*128] bf16 PSUM bank -> a single
        # wide evict into the packed xta buffer, round-robined across
        # DVE / ACT / GPSIMD.
        xta = xtp.tile([P, NK, bc], BF16, tag="xta")
        ups = []
        gt = [None] * nbt  # gt[t]: [128, 6, 128] bf16 (m-major, KF k-tiles)

        def copy_engine(eng, out, in_):
            # PSUM evicts: only DVE / ACT can read PSUM
            if eng % 2 == 0:
                nc.vector.tensor_copy(out, in_)
            else:
                nc.scalar.copy(out, in_)

        def emit_xtt(t):
            # transpose as regular matmuls against identity: the x-tile is
            # the stationary operand, identity streams.  Unlike
            # transpose-mode this pipelines (~LDW-limited) and counts as PE
            # activity for the HAM clock gate.  f32 PSUM (matmul rule)
            # limits each bank to 4 k-tiles.
            eng = t
            for m in range(3):
                kd, base = KD[m], KOFF[m]
                done = 0
                while done < kd:
                    n = min(4, kd - done)
                    tp = pp.tile([P, n * P], F32, tag="ps", name="tp")
                    for j in range(n):
                        nc.tensor.matmul(
                            tp[:, ts(j, P)],
                            lhsT=xn[t][m][:, ts(done + j, P)],
                            rhs=identity,
                            start=True,
                            stop=True,
                        )
                    sl = slice(base + done, base + done + n)
                    copy_engine(
                        eng,
                        xta[:, sl, ts(t, P)],
                        tp.rearrange("p (a b) -> p a b", a=n),
                    )
                    eng += 1
                    done += n

        def emit_proj(t):
            for m in range(3):
                ps = pp.tile([P, D_F], F32, tag="ps", name="ps_proj")
                for k in range(KD[m]):
                    nc.tensor.matmul(
                        ps,
                        lhsT=xta[:, KOFF[m] + k, ts(t, P)],
                        rhs=wc_sb[:, KOFF[m] + k, :],
                        start=(k == 0),
                        stop=(zero_bias and k == KD[m] - 1),
                    )
                if not zero_bias:
                    nc.tensor.matmul(
                        ps, lhsT=ones_row, rhs=bias_sb[:, m, :],
                        start=False, stop=True,
                    )
                stats = sp.tile([P, 6], F32, tag="stats", name="stats")
                nc.vector.bn_stats(stats, ps)
                mv = sp.tile([P, 2], F32, tag="mv", name="mv")
                nc.vector.bn_aggr(mv, stats)
                sd = sp.tile([P, 1], F32, tag="sd", name="sd")
                nc.scalar.activation(sd, mv[:, 1:2], AF.Sqrt, bias=eps_t, scale=1.0)
                rstd = sp.tile([P, 1], F32, tag="rstd", name="rstd")
                nc.vector.reciprocal(rstd, sd)
                u = upool.tile([P, D_F], BF16, tag=f"u{m}", name=f"u{t}_{m}")
                if fast_affine:
                    # u = relu((ps - mu) * rstd) fused on ACT
                    nmu = sp.tile([P, 1], F32, tag="nmu", name="nmu")
                    nc.vector.tensor_scalar(
                        out=nmu,
                        in0=mv[:, 0:1],
                        scalar1=rstd,
                        scalar2=-1.0,
                        op0=OP.mult,
                        op1=OP.mult,
                    )
                    nc.scalar.activation(u, ps, AF.Relu, bias=nmu, scale=rstd)
                else:
                    nc.vector.tensor_scalar(
                        out=u,
                        in0=ps,
                        scalar1=mv[:, 0:1],
                        scalar2=rstd,
                        op0=OP.subtract,
                        op1=OP.mult,
                    )
                ups.append(u)

        def emit_gtt(t):
            g = gtp.tile([P, 6, P], BF16, tag=f"gt{t}", name=f"g{t}")
            for half in range(2):
                tpg = pp.tile([P, 3 * P], F32, tag="ps", name="tpg")
                for i in range(3):
                    col = half * 3 + i
                    m, j = divmod(col, KF)
                    nc.tensor.matmul(
                        tpg[:, ts(i, P)],
                        lhsT=ups[t * 3 + m][:, ts(j, P)],
                        rhs=identity,
                        start=True,
                        stop=True,
                    )
                if fast_affine:
                    copy_engine(
                        t + half,
                        g[:, 3 * half : 3 * half + 3, :],
                        tpg.rearrange("p (a b) -> p a b", a=3),
                    )
                else:
                    for i in range(3):
                        col = half * 3 + i
                        nc.scalar.activation(
                            g[:, col, :],
                            tpg[:, ts(i, P)],
                            AF.Relu,
                            bias=lnb_sb[:, col : col + 1],
                            scale=lnw_sb[:, col : col + 1],
                        )
            gt[t] = g

        emitted_gtt = set()

        def emit_gtt_once(t):
            if t not in emitted_gtt:
                emitted_gtt.add(t)
                emit_gtt(t)

        # ---------- attention layer 1 / softmax / rank ----------
        a1t = wpool.tile([P, NH, bc], BF16, tag="a1t")  # relu(a1)^T
        betas = [None] * nbt
        cpp = tc.tile_pool(name="cp", bufs=3)
        pp2cm = tc.tile_pool(name="ps2", bufs=4, space="PSUM")
        cp = None
        pp2 = None
        in_wave = [False]

        def ps_tile(width, name):
            if not in_wave[0]:
                return pp.tile([P, width], F32, tag="ps", name=name)
            t_ = pp2.tile([P, 1024], F32, tag="ps2", name=name)
            return t_[:, 0:width]

        def emit_attn_h(c, h):
            ps = ps_tile(512, "ps_a1")
            for k in range(NK):
                nc.tensor.matmul(
                    ps,
                    lhsT=wa1_sb[:, k, ts(h, P)],
                    rhs=xta[:, k, ts(c, 512)],
                    start=(k == 0),
                    stop=(k == NK - 1),
                )
            nc.scalar.activation(
                a1t[:, h, ts(c, 512)],
                ps,
                AF.Relu,
                bias=ba1_sb[:, h : h + 1],
                scale=1.0,
            )

        def emit_a2_softmax(t):
            ps = ps_tile(RANK, "ps_a2")
            for k in range(NH):
                nc.tensor.matmul(
                    ps,
                    lhsT=a1t[:, k, ts(t, P)],
                    rhs=wa2_sb[:, k, :],
                    start=(k == 0),
                    stop=(zero_bias and k == NH - 1),
                )
            if not zero_bias:
                nc.tensor.matmul(
                    ps, lhsT=ones_row, rhs=ba2_sb, start=False, stop=True
                )
            # no max-subtraction: logits are bounded (|a2| < ~8), exp is
            # safe in f32, and the beta chain loses a DVE hop
            e = sp.tile([P, RANK], F32, tag="esm", name="esm")
            ssum = sp.tile([P, 1], F32, tag="ssum", name="ssum")
            nc.scalar.activation(e, ps, AF.Exp, scale=1.0, accum_out=ssum)
            rs = sp.tile([P, 1], F32, tag="rs", name="rs")
            nc.vector.reciprocal(rs, ssum)
            beta = gtp.tile([P, RANK], F32, tag=f"beta{t}", name=f"beta{t}")
            nc.vector.tensor_scalar_mul(beta, e, rs)
            betas[t] = beta

        pair_accs = {}

        def emit_rank_pair(ta, tb, c):
            # two tiles' chunks side by side in 4KB two-bank PSUM slots so
            # the trilinear ops run 1024 wide — per-op overhead (~300-600ns
            # on DVE/GPSIMD) was saturating the vector engine at 512
            ps3 = []
            for m in range(3):
                psm = pp2.tile([P, 1024], F32, tag="ps2", name=f"pz{m}")
                for half, t in ((0, ta), (1, tb)):
                    for k in range(KF):
                        nc.tensor.matmul(
                            psm[:, half * 512 : half * 512 + 512],
                            lhsT=gt[t][:, m * KF + k, :],
                            rhs=uvs_sb[:, m * KF + k, ts(c, 512)],
                            start=(k == 0),
                            stop=(k == KF - 1),
                        )
                ps3.append(psm)
            # fold beta into the zg eviction (per-rank scale, on ACT)
            ugb = cp.tile([P, 1024], BF16, tag="ugb", name="ugb")
            for half, t in ((0, ta), (1, tb)):
                for rr in range(2):
                    r = 2 * c + rr
                    lo = half * 512 + rr * D_F
                    nc.scalar.activation(
                        ugb[:, lo : lo + D_F],
                        ps3[0][:, lo : lo + D_F],
                        AF.Copy,
                        scale=betas[t][:, r : r + 1],
                    )
            tm = cp.tile([P, 1024], BF16, tag="tm", name="tm")
            nc.vector.tensor_tensor(tm, ugb, ps3[1], op=OP.mult)
            t2 = cp.tile([P, 1024], BF16, tag="t2", name="t2")
            nc.vector.tensor_tensor(t2, tm, ps3[2], op=OP.mult)
            key = (ta, tb)
            if c == 0:
                acc = zp.tile([P, 1024], F32, tag="acc1024", name="acc1024")
                pair_accs[key] = acc
                nc.gpsimd.tensor_copy(acc, t2)
            else:
                acc = pair_accs[key]
                nc.gpsimd.tensor_tensor(acc, t2, acc, op=OP.add)

        def emit_pair_fin(ta, tb):
            acc = pair_accs[(ta, tb)]
            for half, t in ((0, ta), (1, tb)):
                zfin = zp.tile([P, D_F], F32, tag="zfin", name="zfin")
                eng = nc.gpsimd if half == 0 else nc.vector
                eng.tensor_tensor(
                    zfin,
                    acc[:, half * 512 : half * 512 + D_F],
                    acc[:, half * 512 + D_F : half * 512 + 2 * D_F],
                    op=OP.add,
                )
                nc.sync.dma_start(out=z_out[ts(t, P), :], in_=zfin)

        # ---------- schedule ----------
        # phase A: per-tile [x^T, proj] with the previous tile's g^T
        # trailing one step; attn c0 h-blocks interleave into the tail
        # so the PE never idles between phase A and wave 0.
        for t in range(nbt):
            emit_xtt(t)
            emit_proj(t)
            if t >= 1:
                emit_gtt_once(t - 1)
            if t >= 4 and t - 4 < NH:
                emit_attn_h(0, t - 4)
        for t in range(4):
            emit_a2_softmax(t)
        emit_gtt_once(nbt - 1)
        xnp.__exit__(None, None, None)
        ppcm.__exit__(None, None, None)
        in_wave[0] = True
        pp2 = pp2cm.__enter__()
        cp = cpp.__enter__()

        # wave 0: rank c-major across pairs (01), (23) — the first chunks
        # only need the first uvs slices; attn c1 h-blocks interleave as
        # DVE catch-up windows
        for c in range(NCH):
            emit_rank_pair(0, 1, c)
            emit_rank_pair(2, 3, c)
            if c in (1, 3, 5, 7):
                emit_attn_h(1, c // 2)
        emit_pair_fin(0, 1)
        emit_pair_fin(2, 3)
        for t in range(4, nbt):
            emit_a2_softmax(t)

        # wave 1: pair-sequential
        for c in range(NCH):
            emit_rank_pair(4, 5, c)
        emit_pair_fin(4, 5)
        for c in range(NCH):
            emit_rank_pair(6, 7, c)
        emit_pair_fin(6, 7)
        cpp.__exit__(None, None, None)
        pp2cm.__exit__(None, None, None)

    nc.compile()
    return nc


_BF = ml_dtypes.bfloat16


def _part_major(w, p=P):
    """[T*p, N] -> [p, T*N]: partition-major contiguous packing."""
    t = w.shape[0] // p
    return np.ascontiguousarray(
        w.reshape(t, p, w.shape[1]).transpose(1, 0, 2).reshape(p, -1)
    )


def _pack_weights(inputs):
    """Host-side offline packing: transpose + cast weights once."""
    f = np.asarray
    wc_t = _part_major(
        np.concatenate(
            [f(inputs["Wg"]).T, f(inputs["W2"]).T, f(inputs["W3"]).T], axis=0
        )
    ).astype(_BF)  # [128, 18*256]
    uvs_t = _part_major(
        np.concatenate(
            [f(inputs["U"]).T, f(inputs["V"]).T, f(inputs["S"]).T], axis=0
        )
    ).astype(_BF)  # [128, 6*4096]
    wa1_t = _part_major(
        np.ascontiguousarray(f(inputs["Wa1"]).T)
    ).astype(_BF)  # [128, 18*512]
    wa2_t = _part_major(
        np.ascontiguousarray(f(inputs["Wa2"]).T)
    ).astype(_BF)  # [128, 4*16]
    consts_b = np.zeros((4, D_F), dtype=_BF)
    consts_b[0] = f(inputs["bg"]).astype(_BF)
    consts_b[1] = f(inputs["b2"]).astype(_BF)
    consts_b[2] = f(inputs["b3"]).astype(_BF)
    consts_b[3, :RANK] = f(inputs["ba2"]).astype(_BF)
    consts_f = np.concatenate(
        [
            f(inputs["ba1"]).reshape(NH, P).T,
            np.concatenate(
                [
                    f(inputs["ln_g_w"]).reshape(KF, P),
                    f(inputs["ln_2_w"]).reshape(KF, P),
                    f(inputs["ln_3_w"]).reshape(KF, P),
                ],
                axis=0,
            ).T,
            np.concatenate(
                [
                    f(inputs["ln_g_b"]).reshape(KF, P),
                    f(inputs["ln_2_b"]).reshape(KF, P),
                    f(inputs["ln_3_b"]).reshape(KF, P),
                ],
                axis=0,
            ).T,
        ],
        axis=1,
    ).astype(np.float32)  # [128, 16]
    return {
        "ident": np.eye(P, dtype=_BF),
        "wc_t": wc_t,
        "uvs_t": uvs_t,
        "wa1_t": wa1_t,
        "wa2_t": wa2_t,
        "consts_f": consts_f,
        "consts_b": consts_b,
    }


_NC_CACHE = {}


def _get_nc(fast_affine, zero_bias):
    key = (fast_affine, zero_bias)
    if key not in _NC_CACHE:
        _NC_CACHE[key] = build_kernel(
            fast_affine=fast_affine, zero_bias=zero_bias
        )
    return _NC_CACHE[key]


def kernel(run_opts=None, **inputs):
    f = np.asarray
    fast_affine = all(
        np.all(f(inputs[k]) == 1.0) for k in ("ln_g_w", "ln_2_w", "ln_3_w")
    ) and all(
        np.all(f(inputs[k]) == 0.0) for k in ("ln_g_b", "ln_2_b", "ln_3_b")
    )
    zero_bias = all(
        np.all(f(inputs[k]) == 0.0) for k in ("bg", "b2", "b3", "ba2")
    )
    nc = _get_nc(fast_affine, zero_bias)
    wmap = _pack_weights(inputs)
    h_g = np.ascontiguousarray(np.asarray(inputs["h_g"], dtype=np.float32))
    h_2d = np.ascontiguousarray(np.asarray(inputs["h_2d"], dtype=np.float32))
    h_3d = np.ascontiguousarray(np.asarray(inputs["h_3d"], dtype=np.float32))

    in_maps = []
    for i in range(N_CORES):
        sl = slice(i * BC, (i + 1) * BC)
        m = dict(wmap)
        m["h_g"] = h_g[sl]
        m["h_2d"] = h_2d[sl]
        m["h_3d"] = h_3d[sl]
        in_maps.append(m)

    res = run_bass_kernel_spmd(
        nc, in_maps, core_ids=list(range(N_CORES)), **(run_opts or {})
    )
    out = np.concatenate([r["z"] for r in res.results], axis=0)
    if run_opts:
        kernel.last_results = res
    return out


# revision 29
# speedup vs baseline: 1.0702x; 1.0702x over previous
"""Trainium2 Bass kernel for LowRankTriLinearFusionAttn.

Math (per sample b):
  g  = relu(LN(h_g  @ Wg.T + bg))          (256)
  d2 = relu(LN(h_2d @ W2.T + b2))          (256)
  d3 = relu(LN(h_3d @ W3.T + b3))          (256)
  z_r[b,r,:] = (g U_r^T) * (d2 V_r^T) * (d3 S_r^T)     r in 0..15
  beta = softmax(relu([h_g|h_2d|h_3d] @ Wa1.T + ba1) @ Wa2.T + ba2)
  z[b,:] = sum_r beta[b,r] * z_r[b,r,:]

Sharding: pure data parallel over 8 NeuronCores (batch 8192 -> 1024/core).
Weights are replicated; the host pre-packs them transposed (contraction dim
on partitions) and cast to bf16 — standard offline model packing.
Activations are cast to bf16 by SWDGE DMA and transposed on the PE.

v5 vs the earlier baseline:
- PSUM evicts of the x^T transposes are merged per (tile, modality) into one
  full-width op ([128, KD*128], up to a whole 2KB bank) and round-robined
  across DVE/ACT/GPSIMD instead of 72 narrow ACT copies — the ACT engine was
  the hidden co-bottleneck (69% busy) and its backlog stalled LDWEIGHTS.
- All g^T transposes happen in the projection phase (6 transposes -> one
  PSUM bank -> a single merged evict), so the rank phase never waits on
  weight production.
- With identity LN affine (host-checked), relu folds into the LN-normalize
  eviction on ACT; with zero biases the ones-row bias matmuls are skipped.
- uvs is loaded in 8 rank-chunk slices and wave 0 walks chunks c-major
  (across tiles) so rank work starts while uvs is still streaming.
"""

import sys
import types

import numpy as np
import ml_dtypes

import concourse.bass as bass
import concourse.tile as tile
from concourse import bacc
from concourse import mybir
from concourse.bass import ts
from concourse.bass_utils import run_bass_kernel_spmd
import bass_rust


def _ensure_ntff_hook():
    """Provide antenv.axon_hooks if the image's antenv stub lacks it, so
    run_bass_kernel_spmd(trace=True) can capture NTFF profiles under axon."""
    try:
        import antenv.axon_hooks  # noqa: F401
        return
    except ImportError:
        pass
    try:
        from trn_agent_boot.trn_boot import _ntff_profile_via_ctypes

        hook = _ntff_profile_via_ctypes("/opt/axon/libaxon_pjrt.so")
    except Exception:
        hook = None
    mod = types.ModuleType("antenv.axon_hooks")
    _state = {"hook": hook}
    mod.get_axon_ntff_profile_hook = lambda: _state["hook"]
    mod.set_axon_ntff_profile_hook = lambda h: _state.update(hook=h)
    sys.modules["antenv.axon_hooks"] = mod


_ensure_ntff_hook()

BF16 = mybir.dt.bfloat16
F32 = mybir.dt.float32
AF = mybir.ActivationFunctionType
OP = mybir.AluOpType

N_CORES = 8
B = 8192
D_G, D_2D, D_3D = 512, 768, 1024
D_CAT = D_G + D_2D + D_3D  # 2304
D_F, RANK, ATTN_H = 256, 16, 512
RD = RANK * D_F  # 4096
P = 128

BC = B // N_CORES           # 1024 samples per core
NBT = BC // P               # 8 batch tiles per core
KD = [D_G // P, D_2D // P, D_3D // P]   # k-tiles per modality: 4, 6, 8
KOFF = [0, KD[0], KD[0] + KD[1]]        # xt chunk offsets: 0, 4, 10
NK = D_CAT // P             # 18
NH = ATTN_H // P            # 4
NCH = RD // 512             # 8 chunks of 512 in the rank-expanded dim
KF = D_F // P               # 2 k-tiles for the 256-dim contraction
EPS = 1e-5


def build_kernel(bc=BC, fast_affine=True, zero_bias=True):
    assert bc % 512 == 0
    nbt = bc // P
    nc = bacc.Bacc("TRN2", debug=False)

    # ---- external I/O (per-core shapes) ----
    h_g = nc.dram_tensor("h_g", [bc, D_G], F32, kind="ExternalInput").ap()
    h_2d = nc.dram_tensor("h_2d", [bc, D_2D], F32, kind="ExternalInput").ap()
    h_3d = nc.dram_tensor("h_3d", [bc, D_3D], F32, kind="ExternalInput").ap()
    # weights arrive pre-packed partition-major ([128, ...] contiguous per
    # partition row) so each load is 128 fat contiguous DMA descriptors
    wc_t = nc.dram_tensor("wc_t", [P, NK * D_F], BF16, kind="ExternalInput").ap()
    uvs_t = nc.dram_tensor("uvs_t", [P, 6 * RD], BF16, kind="ExternalInput").ap()
    wa1_t = nc.dram_tensor(
        "wa1_t", [P, NK * ATTN_H], BF16, kind="ExternalInput"
    ).ap()
    wa2_t = nc.dram_tensor("wa2_t", [P, NH * RANK], BF16, kind="ExternalInput").ap()
    consts_f = nc.dram_tensor("consts_f", [P, 16], F32, kind="ExternalInput").ap()
    consts_b = nc.dram_tensor("consts_b", [4, D_F], BF16, kind="ExternalInput").ap()
    ident_in = nc.dram_tensor("ident", [P, P], BF16, kind="ExternalInput").ap()
    z_out = nc.dram_tensor("z", [bc, D_F], F32, kind="ExternalOutput").ap()

    from contextlib import ExitStack

    with tile.TileContext(nc) as tc, ExitStack() as ctx:
        consts = ctx.enter_context(tc.tile_pool(name="consts", bufs=1))
        wpool = ctx.enter_context(tc.tile_pool(name="w", bufs=1))
        xtp = ctx.enter_context(tc.tile_pool(name="xt", bufs=1))
        gtp = ctx.enter_context(tc.tile_pool(name="gt", bufs=1))
        sp = ctx.enter_context(tc.tile_pool(name="sp", bufs=16))
        upool = ctx.enter_context(tc.tile_pool(name="up", bufs=2))
        zp = ctx.enter_context(tc.tile_pool(name="zacc", bufs=4))
        # phase-A PSUM pool: closed before the rank waves, whose pool (pp2,
        # opened later) uses 4KB two-bank slots for tile-paired chunks
        ppcm = tc.tile_pool(name="ps", bufs=6, space="PSUM")
        pp = ppcm.__enter__()
        xnp = tc.tile_pool(name="xn", bufs=4)     # scoped: closed after stage 1
        xnpool = xnp.__enter__()

        # ---------- constants ----------
        identity = consts.tile([P, P], BF16, tag="ident")
        nc.sync.dma_start(out=identity, in_=ident_in)
        ones_row = consts.tile([1, P], BF16, tag="ones")
        nc.vector.memset(ones_row, 1.0)
        eps_t = consts.tile([P, 1], F32, tag="eps")
        nc.vector.memset(eps_t, EPS)
        cf_sb = consts.tile([P, 16], F32, tag="cf")  # ba1(4) lnw(6) lnb(6)
        nc.sync.dma_start(out=cf_sb, in_=consts_f)
        cb_sb = consts.tile([1, 4, D_F], BF16, tag="cb")  # bg b2 b3 ba2pad
        nc.sync.dma_start(out=cb_sb, in_=consts_b.rearrange("(o m) n -> o m n", o=1))
        ba1_sb = cf_sb[:, 0:NH]
        lnw_sb = cf_sb[:, 4:10]
        lnb_sb = cf_sb[:, 10:16]
        bias_sb = cb_sb[:, 0:3, :]
        ba2_sb = cb_sb[:, 3, 0:RANK]

        # ---------- weights (sync queue; wc free, the rest dep-ordered
        # after the input cast stream so inputs win the HBM race) ----------
        wc_sb = wpool.tile([P, NK, D_F], BF16, tag="wc")
        wc_in = wc_t.rearrange("p (t n) -> p t n", t=NK)
        for m in range(3):
            sl = slice(KOFF[m], KOFF[m] + KD[m])
            nc.sync.dma_start(out=wc_sb[:, sl, :], in_=wc_in[:, sl, :])
        wa1_sb = wpool.tile([P, NK, ATTN_H], BF16, tag="wa1")
        d_wa1 = nc.sync.dma_start(
            out=wa1_sb, in_=wa1_t.rearrange("p (t n) -> p t n", t=NK)
        )
        wa2_sb = wpool.tile([P, NH, RANK], BF16, tag="wa2")
        d_wa2 = nc.sync.dma_start(
            out=wa2_sb, in_=wa2_t.rearrange("p (t n) -> p t n", t=NH)
        )
        uvs_sb = wpool.tile([P, 6, RD], BF16, tag="uvs")
        uvs_in = uvs_t.rearrange("p (t n) -> p t n", t=6)
        d_uvs = []
        for c in range(NCH):
            d_uvs.append(
                nc.sync.dma_start(
                    out=uvs_sb[:, :, ts(c, 512)], in_=uvs_in[:, :, ts(c, 512)]
                )
            )

        # ---------- input cast (per (tile, modality) SWDGE cast-DMAs,
        # tile-major so each batch tile is ready after 3 DMAs) ----------
        xin = [h_g, h_2d, h_3d]
        xn = [[None] * 3 for _ in range(nbt)]
        xn_dmas = []
        for t in range(nbt):
            for m in range(3):
                t_ = xnpool.tile(
                    [P, KD[m] * P], BF16, tag=f"xn{m}", name=f"xn{t}_{m}"
                )
                d = nc.gpsimd.dma_start(out=t_, in_=xin[m][ts(t, P), :])
                xn_dmas.append(d)
                xn[t][m] = t_
        # big weights release once tiles 0-3 are through; uvs slices follow
        bass_rust.add_dep_helper(
            d_wa1.ins, xn_dmas[3 * 4 - 1].ins, reason="wa1 after xn tile 3"
        )
        bass_rust.add_dep_helper(d_uvs[0].ins, d_wa1.ins, reason="uvs0 after wa1")
        bass_rust.add_dep_helper(d_wa2.ins, d_uvs[1].ins, reason="wa2 after uvs1")
        bass_rust.add_dep_helper(d_uvs[4].ins, d_wa2.ins, reason="uvs4 after wa2")

        # ---------- HAM warm-up: the PE clock gate opens after ~3.4us of
        # sustained activity; the first real matmuls land ~11us in (DMA
        # preamble + first casts), so burn the idle start on dummy identity
        # matmuls to enter phase A at 2.4GHz instead of 1.2 ----------
        warm = pp.tile([P, P], F32, tag="ps", name="warm")
        for _ in range(56):
            nc.tensor.matmul(warm, lhsT=identity, rhs=identity,
                             start=True, stop=True)

        # ---------- x transposes + projections + LN + gT, per tile -------
        # KD PE transposes fill one [128, KD*128] bf16 PSUM bank -> a single
        # wide evict into the packed xta buffer, round-robined across
        # DVE / ACT / GPSIMD.
        xta = xtp.tile([P, NK, bc], BF16, tag="xta")
        ups = []
        gt = [None] * nbt  # gt[t]: [128, 6, 128] bf16 (m-major, KF k-tiles)

        def copy_engine(eng, out, in_):
            # PSUM evicts: only DVE / ACT can read PSUM
            if eng % 2 == 0:
                nc.vector.tensor_copy(out, in_)
            else:
                nc.scalar.copy(out, in_)

        def emit_xtt(t):
            # transpose as regular matmuls against identity: the x-tile is
            # the stationary operand, identity streams.  Unlike
            # transpose-mode this pipelines (~LDW-limited) and counts as PE
            # activity for the HAM clock gate.  f32 PSUM (matmul rule)
            # limits each bank to 4 k-tiles.
            eng = t
            for m in range(3):
                kd, base = KD[m], KOFF[m]
                done = 0
                while done < kd:
                    n = min(4, kd - done)
                    tp = pp.tile([P, n * P], F32, tag="ps", name="tp")
                    for j in range(n):
                        nc.tensor.matmul(
                            tp[:, ts(j, P)],
                            lhsT=xn[t][m][:, ts(done + j, P)],
                            rhs=identity,
                            start=True,
                            stop=True,
                        )
                    sl = slice(base + done, base + done + n)
                    copy_engine(
                        eng,
                        xta[:, sl, ts(t, P)],
                        tp.rearrange("p (a b) -> p a b", a=n),
                    )
                    eng += 1
                    done += n

        def emit_proj(t):
            for m in range(3):
                ps = pp.tile([P, D_F], F32, tag="ps", name="ps_proj")
                for k in range(KD[m]):
                    nc.tensor.matmul(
                        ps,
                        lhsT=xta[:, KOFF[m] + k, ts(t, P)],
                        rhs=wc_sb[:, KOFF[m] + k, :],
                        start=(k == 0),
                        stop=(zero_bias and k == KD[m] - 1),
                    )
                if not zero_bias:
                    nc.tensor.matmul(
                        ps, lhsT=ones_row, rhs=bias_sb[:, m, :],
                        start=False, stop=True,
                    )
                stats = sp.tile([P, 6], F32, tag="stats", name="stats")
                nc.vector.bn_stats(stats, ps)
                mv = sp.tile([P, 2], F32, tag="mv", name="mv")
                nc.vector.bn_aggr(mv, stats)
                sd = sp.tile([P, 1], F32, tag="sd", name="sd")
                nc.scalar.activation(sd, mv[:, 1:2], AF.Sqrt, bias=eps_t, scale=1.0)
                rstd = sp.tile([P, 1], F32, tag="rstd", name="rstd")
                nc.vector.reciprocal(rstd, sd)
                u = upool.tile([P, D_F], BF16, tag=f"u{m}", name=f"u{t}_{m}")
                if fast_affine:
                    # u = relu((ps - mu) * rstd) fused on ACT
                    nmu = sp.tile([P, 1], F32, tag="nmu", name="nmu")
                    nc.vector.tensor_scalar(
                        out=nmu,
                        in0=mv[:, 0:1],
                        scalar1=rstd,
                        scalar2=-1.0,
                        op0=OP.mult,
                        op1=OP.mult,
                    )
                    nc.scalar.activation(u, ps, AF.Relu, bias=nmu, scale=rstd)
                else:
                    nc.vector.tensor_scalar(
                        out=u,
                        in0=ps,
                        scalar1=mv[:, 0:1],
                        scalar2=rstd,
                        op0=OP.subtract,
                        op1=OP.mult,
                    )
                ups.append(u)

        def emit_gtt(t):
            g = gtp.tile([P, 6, P], BF16, tag=f"gt{t}", name=f"g{t}")
            for half in range(2):
                tpg = pp.tile([P, 3 * P], F32, tag="ps", name="tpg")
                for i in range(3):
                    col = half * 3 + i
                    m, j = divmod(col, KF)
                    nc.tensor.matmul(
                        tpg[:, ts(i, P)],
                        lhsT=ups[t * 3 + m][:, ts(j, P)],
                        rhs=identity,
                        start=True,
                        stop=True,
                    )
                if fast_affine:
                    copy_engine(
                        t + half,
                        g[:, 3 * half : 3 * half + 3, :],
                        tpg.rearrange("p (a b) -> p a b", a=3),
                    )
                else:
                    for i in range(3):
                        col = half * 3 + i
                        nc.scalar.activation(
                            g[:, col, :],
                            tpg[:, ts(i, P)],
                            AF.Relu,
                            bias=lnb_sb[:, col : col + 1],
                            scale=lnw_sb[:, col : col + 1],
                        )
            gt[t] = g

        emitted_gtt = set()

        def emit_gtt_once(t):
            if t not in emitted_gtt:
                emitted_gtt.add(t)
                emit_gtt(t)

        # ---------- attention layer 1 / softmax / rank ----------
        a1t = wpool.tile([P, NH, bc], BF16, tag="a1t")  # relu(a1)^T
        betas = [None] * nbt
        cpp = tc.tile_pool(name="cp", bufs=4)
        pp2cm = tc.tile_pool(name="ps2", bufs=6, space="PSUM")
        ppacm = tc.tile_pool(name="psA", bufs=2, space="PSUM")
        cp = None
        pp2 = None
        ppa = None
        in_wave = [False]

        def ps_tile(width, name):
            if not in_wave[0]:
                return pp.tile([P, width], F32, tag="ps", name=name)
            t_ = ppa.tile([P, 512], F32, tag="psA", name=name)
            return t_[:, 0:width]

        def emit_attn_h(c, h):
            ps = ps_tile(512, "ps_a1")
            for k in range(NK):
                nc.tensor.matmul(
                    ps,
                    lhsT=wa1_sb[:, k, ts(h, P)],
                    rhs=xta[:, k, ts(c, 512)],
                    start=(k == 0),
                    stop=(k == NK - 1),
                )
            nc.scalar.activation(
                a1t[:, h, ts(c, 512)],
                ps,
                AF.Relu,
                bias=ba1_sb[:, h : h + 1],
                scale=1.0,
            )

        def emit_a2_softmax(t):
            ps = ps_tile(RANK, "ps_a2")
            for k in range(NH):
                nc.tensor.matmul(
                    ps,
                    lhsT=a1t[:, k, ts(t, P)],
                    rhs=wa2_sb[:, k, :],
                    start=(k == 0),
                    stop=(zero_bias and k == NH - 1),
                )
            if not zero_bias:
                nc.tensor.matmul(
                    ps, lhsT=ones_row, rhs=ba2_sb, start=False, stop=True
                )
            # no max-subtraction: logits are bounded (|a2| < ~8), exp is
            # safe in f32, and the beta chain loses a DVE hop
            e = sp.tile([P, RANK], F32, tag="esm", name="esm")
            ssum = sp.tile([P, 1], F32, tag="ssum", name="ssum")
            nc.scalar.activation(e, ps, AF.Exp, scale=1.0, accum_out=ssum)
            rs = sp.tile([P, 1], F32, tag="rs", name="rs")
            nc.vector.reciprocal(rs, ssum)
            beta = gtp.tile([P, RANK], F32, tag=f"beta{t}", name=f"beta{t}")
            nc.vector.tensor_scalar_mul(beta, e, rs)
            betas[t] = beta

        accs = [None] * nbt
        accs2 = [None] * nbt

        def emit_rank_chunk(t, c, split_acc=False):
            beta = betas[t]
            pz = []
            for m in range(3):
                ps = pp2.tile([P, 512], F32, tag="ps2", name="ps_rk")
                for k in range(KF):
                    nc.tensor.matmul(
                        ps,
                        lhsT=gt[t][:, m * KF + k, :],
                        rhs=uvs_sb[:, m * KF + k, ts(c, 512)],
                        start=(k == 0),
                        stop=(k == KF - 1),
                    )
                pz.append(ps)
            # fold beta into the zg eviction (per-rank scale, on ACT)
            ugb = cp.tile([P, 512], BF16, tag="ugb", name="ugb")
            for rr in range(2):
                r = 2 * c + rr
                nc.scalar.activation(
                    ugb[:, ts(rr, D_F)],
                    pz[0][:, ts(rr, D_F)],
                    AF.Copy,
                    scale=beta[:, r : r + 1],
                )
            tm = cp.tile([P, 512], BF16, tag="tm", name="tm")
            nc.vector.tensor_tensor(tm, ugb, pz[1], op=OP.mult)
            t2 = cp.tile([P, 512], BF16, tag="t2", name="t2")
            nc.vector.tensor_tensor(t2, tm, pz[2], op=OP.mult)
            if split_acc:
                if c == 0:
                    acc = gtp.tile([P, 512], F32, tag=f"acA{t}", name="accA")
                    accs[t] = acc
                    nc.gpsimd.tensor_copy(acc, t2)
                elif c == 1:
                    acc = gtp.tile([P, 512], F32, tag=f"acB{t}", name="accB")
                    accs2[t] = acc
                    nc.vector.tensor_copy(acc, t2)
                elif c % 2 == 0:
                    nc.gpsimd.tensor_tensor(accs[t], t2, accs[t], op=OP.add)
                else:
                    nc.vector.tensor_tensor(accs2[t], t2, accs2[t], op=OP.add)
            elif c == 0:
                acc = zp.tile([P, 512], F32, tag="acc512", name="acc512")
                accs[t] = acc
                nc.gpsimd.tensor_copy(acc, t2)
            else:
                nc.gpsimd.tensor_tensor(accs[t], t2, accs[t], op=OP.add)

        def emit_rank_fin(t):
            if accs2[t] is not None:
                both = gtp.tile([P, 512], F32, tag=f"abo{t}", name="abo")
                nc.vector.tensor_tensor(both, accs[t], accs2[t], op=OP.add)
                acc = both
            else:
                acc = accs[t]
            zfin = zp.tile([P, D_F], F32, tag="zfin", name="zfin")
            nc.gpsimd.tensor_tensor(
                zfin, acc[:, 0:D_F], acc[:, D_F : 2 * D_F], op=OP.add
            )
            nc.sync.dma_start(out=z_out[ts(t, P), :], in_=zfin)

        # ---------- schedule ----------
        for t in range(nbt):
            emit_xtt(t)
            emit_proj(t)
            if t >= 1:
                emit_gtt_once(t - 1)
            if t >= 4 and t - 4 < NH:
                emit_attn_h(0, t - 4)
        for t in range(4):
            emit_a2_softmax(t)
        emit_gtt_once(nbt - 1)
        xnp.__exit__(None, None, None)
        ppcm.__exit__(None, None, None)
        in_wave[0] = True
        ppa = ppacm.__enter__()
        pp2 = pp2cm.__enter__()
        cp = cpp.__enter__()

        # wave 0: rank c-major across tiles 0-3; attn c1 h-blocks
        # interleave as DVE catch-up windows
        for c in range(NCH):
            for t in range(4):
                emit_rank_chunk(t, c)
            if c in (1, 3, 5, 7):
                emit_attn_h(1, c // 2)
        for t in range(4):
            emit_rank_fin(t)
        for t in range(4, nbt):
            emit_a2_softmax(t)

        # wave 1: t-major for 4,5; chunk-interleaved split-acc tail for 6,7
        for t in (4, 5):
            for c in range(NCH):
                emit_rank_chunk(t, c)
            emit_rank_fin(t)
        for c in range(NCH):
            emit_rank_chunk(6, c)
            emit_rank_chunk(7, c, split_acc=True)
        emit_rank_fin(6)
        emit_rank_fin(7)
        cpp.__exit__(None, None, None)
        pp2cm.__exit__(None, None, None)
        ppacm.__exit__(None, None, None)

    nc.compile()
    return nc


_BF = ml_dtypes.bfloat16


def _part_major(w, p=P):
    """[T*p, N] -> [p, T*N]: partition-major contiguous packing."""
    t = w.shape[0] // p
    return np.ascontiguousarray(
        w.reshape(t, p, w.shape[1]).transpose(1, 0, 2).reshape(p, -1)
    )


def _pack_weights(inputs):
    """Host-side offline packing: transpose + cast weights once."""
    f = np.asarray
    wc_t = _part_major(
        np.concatenate(
            [f(inputs["Wg"]).T, f(inputs["W2"]).T, f(inputs["W3"]).T], axis=0
        )
    ).astype(_BF)  # [128, 18*256]
    uvs_t = _part_major(
        np.concatenate(
            [f(inputs["U"]).T, f(inputs["V"]).T, f(inputs["S"]).T], axis=0
        )
    ).astype(_BF)  # [128, 6*4096]
    wa1_t = _part_major(
        np.ascontiguousarray(f(inputs["Wa1"]).T)
    ).astype(_BF)  # [128, 18*512]
    wa2_t = _part_major(
        np.ascontiguousarray(f(inputs["Wa2"]).T)
    ).astype(_BF)  # [128, 4*16]
    consts_b = np.zeros((4, D_F), dtype=_BF)
    consts_b[0] = f(inputs["bg"]).astype(_BF)
    consts_b[1] = f(inputs["b2"]).astype(_BF)
    consts_b[2] = f(inputs["b3"]).astype(_BF)
    consts_b[3, :RANK] = f(inputs["ba2"]).astype(_BF)
    consts_f = np.concatenate(
        [
            f(inputs["ba1"]).reshape(NH, P).T,
            np.concatenate(
                [
                    f(inputs["ln_g_w"]).reshape(KF, P),
                    f(inputs["ln_2_w"]).reshape(KF, P),
                    f(inputs["ln_3_w"]).reshape(KF, P),
                ],
                axis=0,
            ).T,
            np.concatenate(
                [
                    f(inputs["ln_g_b"]).reshape(KF, P),
                    f(inputs["ln_2_b"]).reshape(KF, P),
                    f(inputs["ln_3_b"]).reshape(KF, P),
                ],
                axis=0,
            ).T,
        ],
        axis=1,
    ).astype(np.float32)  # [128, 16]
    return {
        "ident": np.eye(P, dtype=_BF),
        "wc_t": wc_t,
        "uvs_t": uvs_t,
        "wa1_t": wa1_t,
        "wa2_t": wa2_t,
        "consts_f": consts_f,
        "consts_b": consts_b,
    }


_NC_CACHE = {}


def _get_nc(fast_affine, zero_bias):
    key = (fast_affine, zero_bias)
    if key not in _NC_CACHE:
        _NC_CACHE[key] = build_kernel(
            fast_affine=fast_affine, zero_bias=zero_bias
        )
    return _NC_CACHE[key]


def kernel(run_opts=None, **inputs):
    f = np.asarray
    fast_affine = all(
        np.all(f(inputs[k]) == 1.0) for k in ("ln_g_w", "ln_2_w", "ln_3_w")
    ) and all(
        np.all(f(inputs[k]) == 0.0) for k in ("ln_g_b", "ln_2_b", "ln_3_b")
    )
    zero_bias = all(
        np.all(f(inputs[k]) == 0.0) for k in ("bg", "b2", "b3", "ba2")
    )
    nc = _get_nc(fast_affine, zero_bias)
    wmap = _pack_weights(inputs)
    h_g = np.ascontiguousarray(np.asarray(inputs["h_g"], dtype=np.float32))
    h_2d = np.ascontiguousarray(np.asarray(inputs["h_2d"], dtype=np.float32))
    h_3d = np.ascontiguousarray(np.asarray(inputs["h_3d"], dtype=np.float32))

    in_maps = []
    for i in range(N_CORES):
        sl = slice(i * BC, (i + 1) * BC)
        m = dict(wmap)
        m["h_g"] = h_g[sl]
        m["h_2d"] = h_2d[sl]
        m["h_3d"] = h_3d[sl]
        in_maps.append(m)

    res = run_bass_kernel_spmd(
        nc, in_maps, core_ids=list(range(N_CORES)), **(run_opts or {})
    )
    out = np.concatenate([r["z"] for r in res.results], axis=0)
    if run_opts:
        kernel.last_results = res
    return out


# revision 30
# speedup vs baseline: 1.1021x; 1.0298x over previous
"""Trainium2 Bass kernel for LowRankTriLinearFusionAttn.

Math (per sample b):
  g  = relu(LN(h_g  @ Wg.T + bg))          (256)
  d2 = relu(LN(h_2d @ W2.T + b2))          (256)
  d3 = relu(LN(h_3d @ W3.T + b3))          (256)
  z_r[b,r,:] = (g U_r^T) * (d2 V_r^T) * (d3 S_r^T)     r in 0..15
  beta = softmax(relu([h_g|h_2d|h_3d] @ Wa1.T + ba1) @ Wa2.T + ba2)
  z[b,:] = sum_r beta[b,r] * z_r[b,r,:]

Sharding: pure data parallel over 8 NeuronCores (batch 8192 -> 1024/core).
Weights are replicated; the host pre-packs them transposed (contraction dim
on partitions) and cast to bf16 — standard offline model packing.
Activations are cast to bf16 by SWDGE DMA and transposed on the PE.

v5 vs the earlier baseline:
- PSUM evicts of the x^T transposes are merged per (tile, modality) into one
  full-width op ([128, KD*128], up to a whole 2KB bank) and round-robined
  across DVE/ACT/GPSIMD instead of 72 narrow ACT copies — the ACT engine was
  the hidden co-bottleneck (69% busy) and its backlog stalled LDWEIGHTS.
- All g^T transposes happen in the projection phase (6 transposes -> one
  PSUM bank -> a single merged evict), so the rank phase never waits on
  weight production.
- With identity LN affine (host-checked), relu folds into the LN-normalize
  eviction on ACT; with zero biases the ones-row bias matmuls are skipped.
- uvs is loaded in 8 rank-chunk slices and wave 0 walks chunks c-major
  (across tiles) so rank work starts while uvs is still streaming.
"""

import sys
import types

import numpy as np
import ml_dtypes

import concourse.bass as bass
import concourse.tile as tile
from concourse import bacc
from concourse import mybir
from concourse.bass import ts
from concourse.bass_utils import run_bass_kernel_spmd
import bass_rust


def _ensure_ntff_hook():
    """Provide antenv.axon_hooks if the image's antenv stub lacks it, so
    run_bass_kernel_spmd(trace=True) can capture NTFF profiles under axon."""
    try:
        import antenv.axon_hooks  # noqa: F401
        return
    except ImportError:
        pass
    try:
        from trn_agent_boot.trn_boot import _ntff_profile_via_ctypes

        hook = _ntff_profile_via_ctypes("/opt/axon/libaxon_pjrt.so")
    except Exception:
        hook = None
    mod = types.ModuleType("antenv.axon_hooks")
    _state = {"hook": hook}
    mod.get_axon_ntff_profile_hook = lambda: _state["hook"]
    mod.set_axon_ntff_profile_hook = lambda h: _state.update(hook=h)
    sys.modules["antenv.axon_hooks"] = mod


_ensure_ntff_hook()

BF16 = mybir.dt.bfloat16
F32 = mybir.dt.float32
AF = mybir.ActivationFunctionType
OP = mybir.AluOpType

N_CORES = 8
B = 8192
D_G, D_2D, D_3D = 512, 768, 1024
D_CAT = D_G + D_2D + D_3D  # 2304
D_F, RANK, ATTN_H = 256, 16, 512
RD = RANK * D_F  # 4096
P = 128

BC = B // N_CORES           # 1024 samples per core
NBT = BC // P               # 8 batch tiles per core
KD = [D_G // P, D_2D // P, D_3D // P]   # k-tiles per modality: 4, 6, 8
KOFF = [0, KD[0], KD[0] + KD[1]]        # xt chunk offsets: 0, 4, 10
NK = D_CAT // P             # 18
NH = ATTN_H // P            # 4
NCH = RD // 512             # 8 chunks of 512 in the rank-expanded dim
KF = D_F // P               # 2 k-tiles for the 256-dim contraction
EPS = 1e-5


def build_kernel(bc=BC, fast_affine=True, zero_bias=True):
    assert bc % 512 == 0
    nbt = bc // P
    nc = bacc.Bacc("TRN2", debug=False)

    # ---- external I/O (per-core shapes) ----
    h_g = nc.dram_tensor("h_g", [bc, D_G], F32, kind="ExternalInput").ap()
    h_2d = nc.dram_tensor("h_2d", [bc, D_2D], F32, kind="ExternalInput").ap()
    h_3d = nc.dram_tensor("h_3d", [bc, D_3D], F32, kind="ExternalInput").ap()
    # weights arrive pre-packed partition-major ([128, ...] contiguous per
    # partition row) so each load is 128 fat contiguous DMA descriptors
    wc_t = nc.dram_tensor("wc_t", [P, NK * D_F], BF16, kind="ExternalInput").ap()
    uvs_t = nc.dram_tensor("uvs_t", [P, 6 * RD], BF16, kind="ExternalInput").ap()
    wa1_t = nc.dram_tensor(
        "wa1_t", [P, NK * ATTN_H], BF16, kind="ExternalInput"
    ).ap()
    wa2_t = nc.dram_tensor("wa2_t", [P, NH * RANK], BF16, kind="ExternalInput").ap()
    consts_f = nc.dram_tensor("consts_f", [P, 16], F32, kind="ExternalInput").ap()
    consts_b = nc.dram_tensor("consts_b", [4, D_F], BF16, kind="ExternalInput").ap()
    ident_in = nc.dram_tensor("ident", [P, P], BF16, kind="ExternalInput").ap()
    z_out = nc.dram_tensor("z", [bc, D_F], F32, kind="ExternalOutput").ap()

    from contextlib import ExitStack

    with tile.TileContext(nc) as tc, ExitStack() as ctx:
        consts = ctx.enter_context(tc.tile_pool(name="consts", bufs=1))
        wpool = ctx.enter_context(tc.tile_pool(name="w", bufs=1))
        xtp = ctx.enter_context(tc.tile_pool(name="xt", bufs=1))
        gtp = ctx.enter_context(tc.tile_pool(name="gt", bufs=1))
        sp = ctx.enter_context(tc.tile_pool(name="sp", bufs=16))
        upool = ctx.enter_context(tc.tile_pool(name="up", bufs=2))
        zp = ctx.enter_context(tc.tile_pool(name="zacc", bufs=4))
        pp = ctx.enter_context(tc.tile_pool(name="ps", bufs=8, space="PSUM"))
        xnp = tc.tile_pool(name="xn", bufs=4)     # scoped: closed after stage 1
        xnpool = xnp.__enter__()

        # ---------- constants ----------
        identity = consts.tile([P, P], BF16, tag="ident")
        nc.sync.dma_start(out=identity, in_=ident_in)
        ones_row = consts.tile([1, P], BF16, tag="ones")
        nc.vector.memset(ones_row, 1.0)
        eps_t = consts.tile([P, 1], F32, tag="eps")
        nc.vector.memset(eps_t, EPS)
        cf_sb = consts.tile([P, 16], F32, tag="cf")  # ba1(4) lnw(6) lnb(6)
        nc.sync.dma_start(out=cf_sb, in_=consts_f)
        cb_sb = consts.tile([1, 4, D_F], BF16, tag="cb")  # bg b2 b3 ba2pad
        nc.sync.dma_start(out=cb_sb, in_=consts_b.rearrange("(o m) n -> o m n", o=1))
        ba1_sb = cf_sb[:, 0:NH]
        lnw_sb = cf_sb[:, 4:10]
        lnb_sb = cf_sb[:, 10:16]
        bias_sb = cb_sb[:, 0:3, :]
        ba2_sb = cb_sb[:, 3, 0:RANK]

        # ---------- weights (sync queue; wc free, the rest dep-ordered
        # after the input cast stream so inputs win the HBM race) ----------
        wc_sb = wpool.tile([P, NK, D_F], BF16, tag="wc")
        wc_in = wc_t.rearrange("p (t n) -> p t n", t=NK)
        for m in range(3):
            sl = slice(KOFF[m], KOFF[m] + KD[m])
            nc.sync.dma_start(out=wc_sb[:, sl, :], in_=wc_in[:, sl, :])
        wa1_sb = wpool.tile([P, NK, ATTN_H], BF16, tag="wa1")
        d_wa1 = nc.sync.dma_start(
            out=wa1_sb, in_=wa1_t.rearrange("p (t n) -> p t n", t=NK)
        )
        wa2_sb = wpool.tile([P, NH, RANK], BF16, tag="wa2")
        d_wa2 = nc.sync.dma_start(
            out=wa2_sb, in_=wa2_t.rearrange("p (t n) -> p t n", t=NH)
        )
        uvs_sb = wpool.tile([P, 6, RD], BF16, tag="uvs")
        uvs_in = uvs_t.rearrange("p (t n) -> p t n", t=6)
        d_uvs = []
        for c in range(NCH):
            d_uvs.append(
                nc.sync.dma_start(
                    out=uvs_sb[:, :, ts(c, 512)], in_=uvs_in[:, :, ts(c, 512)]
                )
            )

        # ---------- input cast (per (tile, modality) SWDGE cast-DMAs,
        # tile-major so each batch tile is ready after 3 DMAs) ----------
        xin = [h_g, h_2d, h_3d]
        xn = [[None] * 3 for _ in range(nbt)]
        xn_dmas = []
        for t in range(nbt):
            for m in range(3):
                t_ = xnpool.tile(
                    [P, KD[m] * P], BF16, tag=f"xn{m}", name=f"xn{t}_{m}"
                )
                d = nc.gpsimd.dma_start(out=t_, in_=xin[m][ts(t, P), :])
                xn_dmas.append(d)
                xn[t][m] = t_
        # big weights release once tiles 0-3 are through; uvs slices follow
        bass_rust.add_dep_helper(
            d_wa1.ins, xn_dmas[3 * 4 - 1].ins, reason="wa1 after xn tile 3"
        )
        bass_rust.add_dep_helper(d_uvs[0].ins, d_wa1.ins, reason="uvs0 after wa1")
        bass_rust.add_dep_helper(d_wa2.ins, d_uvs[1].ins, reason="wa2 after uvs1")
        bass_rust.add_dep_helper(d_uvs[4].ins, d_wa2.ins, reason="uvs4 after wa2")

        # ---------- x transposes + projections + LN + gT, per tile -------
        # KD PE transposes fill one [128, KD*128] bf16 PSUM bank -> a single
        # wide evict into the packed xta buffer, round-robined across
        # DVE / ACT / GPSIMD.
        xta = xtp.tile([P, NK, bc], BF16, tag="xta")
        ups = []
        gt = [None] * nbt  # gt[t]: [128, 6, 128] bf16 (m-major, KF k-tiles)

        def copy_engine(eng, out, in_):
            # PSUM evicts: only DVE / ACT can read PSUM
            if eng % 2 == 0:
                nc.vector.tensor_copy(out, in_)
            else:
                nc.scalar.copy(out, in_)

        def emit_xtt(t):
            # transpose as regular matmuls against identity: the x-tile is
            # the stationary operand, identity streams.  Unlike
            # transpose-mode this pipelines (~LDW-limited) and counts as PE
            # activity for the HAM clock gate.  f32 PSUM (matmul rule)
            # limits each bank to 4 k-tiles.
            eng = t
            for m in range(3):
                kd, base = KD[m], KOFF[m]
                done = 0
                while done < kd:
                    n = min(4, kd - done)
                    tp = pp.tile([P, n * P], F32, tag="ps", name="tp")
                    for j in range(n):
                        nc.tensor.matmul(
                            tp[:, ts(j, P)],
                            lhsT=xn[t][m][:, ts(done + j, P)],
                            rhs=identity,
                            start=True,
                            stop=True,
                        )
                    sl = slice(base + done, base + done + n)
                    copy_engine(
                        eng,
                        xta[:, sl, ts(t, P)],
                        tp.rearrange("p (a b) -> p a b", a=n),
                    )
                    eng += 1
                    done += n

        def emit_proj(t):
            for m in range(3):
                ps = pp.tile([P, D_F], F32, tag="ps", name="ps_proj")
                for k in range(KD[m]):
                    nc.tensor.matmul(
                        ps,
                        lhsT=xta[:, KOFF[m] + k, ts(t, P)],
                        rhs=wc_sb[:, KOFF[m] + k, :],
                        start=(k == 0),
                        stop=(zero_bias and k == KD[m] - 1),
                    )
                if not zero_bias:
                    nc.tensor.matmul(
                        ps, lhsT=ones_row, rhs=bias_sb[:, m, :],
                        start=False, stop=True,
                    )
                stats = sp.tile([P, 6], F32, tag="stats", name="stats")
                nc.vector.bn_stats(stats, ps)
                mv = sp.tile([P, 2], F32, tag="mv", name="mv")
                nc.vector.bn_aggr(mv, stats)
                sd = sp.tile([P, 1], F32, tag="sd", name="sd")
                nc.scalar.activation(sd, mv[:, 1:2], AF.Sqrt, bias=eps_t, scale=1.0)
                rstd = sp.tile([P, 1], F32, tag="rstd", name="rstd")
                nc.vector.reciprocal(rstd, sd)
                u = upool.tile([P, D_F], BF16, tag=f"u{m}", name=f"u{t}_{m}")
                if fast_affine:
                    # u = relu((ps - mu) * rstd) fused on ACT
                    nmu = sp.tile([P, 1], F32, tag="nmu", name="nmu")
                    nc.vector.tensor_scalar(
                        out=nmu,
                        in0=mv[:, 0:1],
                        scalar1=rstd,
                        scalar2=-1.0,
                        op0=OP.mult,
                        op1=OP.mult,
                    )
                    nc.scalar.activation(u, ps, AF.Relu, bias=nmu, scale=rstd)
                else:
                    nc.vector.tensor_scalar(
                        out=u,
                        in0=ps,
                        scalar1=mv[:, 0:1],
                        scalar2=rstd,
                        op0=OP.subtract,
                        op1=OP.mult,
                    )
                ups.append(u)

        def emit_gtt(t):
            g = gtp.tile([P, 6, P], BF16, tag=f"gt{t}", name=f"g{t}")
            for half in range(2):
                tpg = pp.tile([P, 3 * P], F32, tag="ps", name="tpg")
                for i in range(3):
                    col = half * 3 + i
                    m, j = divmod(col, KF)
                    nc.tensor.matmul(
                        tpg[:, ts(i, P)],
                        lhsT=ups[t * 3 + m][:, ts(j, P)],
                        rhs=identity,
                        start=True,
                        stop=True,
                    )
                if fast_affine:
                    copy_engine(
                        t + half,
                        g[:, 3 * half : 3 * half + 3, :],
                        tpg.rearrange("p (a b) -> p a b", a=3),
                    )
                else:
                    for i in range(3):
                        col = half * 3 + i
                        nc.scalar.activation(
                            g[:, col, :],
                            tpg[:, ts(i, P)],
                            AF.Relu,
                            bias=lnb_sb[:, col : col + 1],
                            scale=lnw_sb[:, col : col + 1],
                        )
            gt[t] = g

        emitted_gtt = set()

        def emit_gtt_once(t):
            if t not in emitted_gtt:
                emitted_gtt.add(t)
                emit_gtt(t)

        # ---------- attention layer 1 / softmax / rank ----------
        a1t = wpool.tile([P, NH, bc], BF16, tag="a1t")  # relu(a1)^T
        betas = [None] * nbt
        cpp = tc.tile_pool(name="cp", bufs=4)
        cp = None

        def ps_tile(width, name):
            return pp.tile([P, width], F32, tag="ps", name=name)

        def emit_attn_h(c, h):
            ps = ps_tile(512, "ps_a1")
            for k in range(NK):
                nc.tensor.matmul(
                    ps,
                    lhsT=wa1_sb[:, k, ts(h, P)],
                    rhs=xta[:, k, ts(c, 512)],
                    start=(k == 0),
                    stop=(k == NK - 1),
                )
            nc.scalar.activation(
                a1t[:, h, ts(c, 512)],
                ps,
                AF.Relu,
                bias=ba1_sb[:, h : h + 1],
                scale=1.0,
            )

        def emit_a2_softmax(t):
            ps = ps_tile(RANK, "ps_a2")
            for k in range(NH):
                nc.tensor.matmul(
                    ps,
                    lhsT=a1t[:, k, ts(t, P)],
                    rhs=wa2_sb[:, k, :],
                    start=(k == 0),
                    stop=(zero_bias and k == NH - 1),
                )
            if not zero_bias:
                nc.tensor.matmul(
                    ps, lhsT=ones_row, rhs=ba2_sb, start=False, stop=True
                )
            # no max-subtraction: logits are bounded (|a2| < ~8), exp is
            # safe in f32, and the beta chain loses a DVE hop
            e = sp.tile([P, RANK], F32, tag="esm", name="esm")
            ssum = sp.tile([P, 1], F32, tag="ssum", name="ssum")
            nc.scalar.activation(e, ps, AF.Exp, scale=1.0, accum_out=ssum)
            rs = sp.tile([P, 1], F32, tag="rs", name="rs")
            nc.vector.reciprocal(rs, ssum)
            beta = gtp.tile([P, RANK], F32, tag=f"beta{t}", name=f"beta{t}")
            nc.vector.tensor_scalar_mul(beta, e, rs)
            betas[t] = beta

        accs = [None] * nbt
        accs2 = [None] * nbt

        def emit_rank_chunk(t, c, split_acc=False):
            beta = betas[t]
            pz = []
            for m in range(3):
                ps = pp.tile([P, 512], F32, tag="ps", name="ps_rk")
                for k in range(KF):
                    nc.tensor.matmul(
                        ps,
                        lhsT=gt[t][:, m * KF + k, :],
                        rhs=uvs_sb[:, m * KF + k, ts(c, 512)],
                        start=(k == 0),
                        stop=(k == KF - 1),
                    )
                pz.append(ps)
            # fold beta into the zg eviction (per-rank scale, on ACT)
            ugb = cp.tile([P, 512], BF16, tag="ugb", name="ugb")
            for rr in range(2):
                r = 2 * c + rr
                nc.scalar.activation(
                    ugb[:, ts(rr, D_F)],
                    pz[0][:, ts(rr, D_F)],
                    AF.Copy,
                    scale=beta[:, r : r + 1],
                )
            tm = cp.tile([P, 512], BF16, tag="tm", name="tm")
            nc.vector.tensor_tensor(tm, ugb, pz[1], op=OP.mult)
            t2 = cp.tile([P, 512], BF16, tag="t2", name="t2")
            nc.vector.tensor_tensor(t2, tm, pz[2], op=OP.mult)
            if split_acc:
                if c == 0:
                    acc = gtp.tile([P, 512], F32, tag=f"acA{t}", name="accA")
                    accs[t] = acc
                    nc.gpsimd.tensor_copy(acc, t2)
                elif c == 1:
                    acc = gtp.tile([P, 512], F32, tag=f"acB{t}", name="accB")
                    accs2[t] = acc
                    nc.vector.tensor_copy(acc, t2)
                elif c % 2 == 0:
                    nc.gpsimd.tensor_tensor(accs[t], t2, accs[t], op=OP.add)
                else:
                    nc.vector.tensor_tensor(accs2[t], t2, accs2[t], op=OP.add)
            elif c == 0:
                acc = zp.tile([P, 512], F32, tag="acc512", name="acc512")
                accs[t] = acc
                nc.gpsimd.tensor_copy(acc, t2)
            else:
                nc.gpsimd.tensor_tensor(accs[t], t2, accs[t], op=OP.add)

        def emit_rank_fin(t):
            if accs2[t] is not None:
                both = gtp.tile([P, 512], F32, tag=f"abo{t}", name="abo")
                nc.vector.tensor_tensor(both, accs[t], accs2[t], op=OP.add)
                acc = both
            else:
                acc = accs[t]
            zfin = zp.tile([P, D_F], F32, tag="zfin", name="zfin")
            nc.gpsimd.tensor_tensor(
                zfin, acc[:, 0:D_F], acc[:, D_F : 2 * D_F], op=OP.add
            )
            nc.sync.dma_start(out=z_out[ts(t, P), :], in_=zfin)

        # ---------- schedule ----------
        for t in range(nbt):
            emit_xtt(t)
            emit_proj(t)
            if t >= 1:
                emit_gtt_once(t - 1)
            if t >= 4 and t - 4 < NH:
                emit_attn_h(0, t - 4)
        for t in range(4):
            emit_a2_softmax(t)
        emit_gtt_once(nbt - 1)
        xnp.__exit__(None, None, None)
        cp = cpp.__enter__()

        # wave 0: rank c-major across tiles 0-3; attn c1 h-blocks
        # interleave as DVE catch-up windows
        for c in range(NCH):
            for t in range(4):
                emit_rank_chunk(t, c)
            if c in (1, 3, 5, 7):
                emit_attn_h(1, c // 2)
        for t in range(4):
            emit_rank_fin(t)
        for t in range(4, nbt):
            emit_a2_softmax(t)

        # wave 1: t-major for 4,5; chunk-interleaved split-acc tail for 6,7
        for t in (4, 5):
            for c in range(NCH):
                emit_rank_chunk(t, c)
            emit_rank_fin(t)
        for c in range(NCH):
            emit_rank_chunk(6, c, split_acc=True)
            emit_rank_chunk(7, c, split_acc=True)
        emit_rank_fin(6)
        emit_rank_fin(7)
        cpp.__exit__(None, None, None)

    nc.compile()
    return nc


_BF = ml_dtypes.bfloat16


def _part_major(w, p=P):
    """[T*p, N] -> [p, T*N]: partition-major contiguous packing."""
    t = w.shape[0] // p
    return np.ascontiguousarray(
        w.reshape(t, p, w.shape[1]).transpose(1, 0, 2).reshape(p, -1)
    )


def _pack_weights(inputs):
    """Host-side offline packing: transpose + cast weights once."""
    f = np.asarray
    wc_t = _part_major(
        np.concatenate(
            [f(inputs["Wg"]).T, f(inputs["W2"]).T, f(inputs["W3"]).T], axis=0
        )
    ).astype(_BF)  # [128, 18*256]
    uvs_t = _part_major(
        np.concatenate(
            [f(inputs["U"]).T, f(inputs["V"]).T, f(inputs["S"]).T], axis=0
        )
    ).astype(_BF)  # [128, 6*4096]
    wa1_t = _part_major(
        np.ascontiguousarray(f(inputs["Wa1"]).T)
    ).astype(_BF)  # [128, 18*512]
    wa2_t = _part_major(
        np.ascontiguousarray(f(inputs["Wa2"]).T)
    ).astype(_BF)  # [128, 4*16]
    consts_b = np.zeros((4, D_F), dtype=_BF)
    consts_b[0] = f(inputs["bg"]).astype(_BF)
    consts_b[1] = f(inputs["b2"]).astype(_BF)
    consts_b[2] = f(inputs["b3"]).astype(_BF)
    consts_b[3, :RANK] = f(inputs["ba2"]).astype(_BF)
    consts_f = np.concatenate(
        [
            f(inputs["ba1"]).reshape(NH, P).T,
            np.concatenate(
                [
                    f(inputs["ln_g_w"]).reshape(KF, P),
                    f(inputs["ln_2_w"]).reshape(KF, P),
                    f(inputs["ln_3_w"]).reshape(KF, P),
                ],
                axis=0,
            ).T,
            np.concatenate(
                [
                    f(inputs["ln_g_b"]).reshape(KF, P),
                    f(inputs["ln_2_b"]).reshape(KF, P),
                    f(inputs["ln_3_b"]).reshape(KF, P),
                ],
                axis=0,
            ).T,
        ],
        axis=1,
    ).astype(np.float32)  # [128, 16]
    return {
        "ident": np.eye(P, dtype=_BF),
        "wc_t": wc_t,
        "uvs_t": uvs_t,
        "wa1_t": wa1_t,
        "wa2_t": wa2_t,
        "consts_f": consts_f,
        "consts_b": consts_b,
    }


_NC_CACHE = {}


def _get_nc(fast_affine, zero_bias):
    key = (fast_affine, zero_bias)
    if key not in _NC_CACHE:
        _NC_CACHE[key] = build_kernel(
            fast_affine=fast_affine, zero_bias=zero_bias
        )
    return _NC_CACHE[key]


def kernel(run_opts=None, **inputs):
    f = np.asarray
    fast_affine = all(
        np.all(f(inputs[k]) == 1.0) for k in ("ln_g_w", "ln_2_w", "ln_3_w")
    ) and all(
        np.all(f(inputs[k]) == 0.0) for k in ("ln_g_b", "ln_2_b", "ln_3_b")
    )
    zero_bias = all(
        np.all(f(inputs[k]) == 0.0) for k in ("bg", "b2", "b3", "ba2")
    )
    nc = _get_nc(fast_affine, zero_bias)
    wmap = _pack_weights(inputs)
    h_g = np.ascontiguousarray(np.asarray(inputs["h_g"], dtype=np.float32))
    h_2d = np.ascontiguousarray(np.asarray(inputs["h_2d"], dtype=np.float32))
    h_3d = np.ascontiguousarray(np.asarray(inputs["h_3d"], dtype=np.float32))

    in_maps = []
    for i in range(N_CORES):
        sl = slice(i * BC, (i + 1) * BC)
        m = dict(wmap)
        m["h_g"] = h_g[sl]
        m["h_2d"] = h_2d[sl]
        m["h_3d"] = h_3d[sl]
        in_maps.append(m)

    res = run_bass_kernel_spmd(
        nc, in_maps, core_ids=list(range(N_CORES)), **(run_opts or {})
    )
    out = np.concatenate([r["z"] for r in res.results], axis=0)
    if run_opts:
        kernel.last_results = res
    return out
